# revision 1
# baseline (speedup 1.0000x reference)
"""EnhancedTransformerBlock on 8 TRN2 NeuronCores.

Strategy: pure data-parallel over batch (B=8 -> 1 element/core, no
collectives). Per core the block runs in "T-layout" ([feature, token],
features on partitions) so every matmul contracts over the partition dim.
bf16 matmul inputs (weights pre-cast on host; activations cast at
psum-evict), fp32 PSUM accumulate and fp32 trunk (residuals, LN stats,
softmax normalization). x is transposed on the PE at startup (f32
identity-matmul transposes while the PE is otherwise idle), evicted
both as fp32 (residual path) and bf16 (matmul path).

Softmax: scores^T = K_h Q_h^T per head (no max subtraction needed -- the
logits are O(1)), exp on ACT straight out of PSUM, row-sums via an
appended ones-column on V (row 64 of the AV matmul output), normalization
by a K=1 "broadcast" matmul of the reciprocal row. Heads are processed in
pairs mapped to PE row-groups 0-1/2-3 so their K=64 score matmuls overlap.

LayerNorm1 in T-layout: column sums via ones-column matmuls, per-token
scale/shift broadcast across partitions with K=1 matmuls. LayerNorm2 in
N-layout after PE f32 transposes (bn_stats/bn_aggr), writing the output
directly in [token, feature] order.

SBUF slabs are tag-shared across phases (QTs->gT, xT_bf->ctxT, Vp->rT,
QTs..): same tag + bufs=1 reuses the address, Tile's WAR tracking orders
the phases.
"""
import sys

sys.path.insert(0, '/opt/trn_rl_repo')

import numpy as np
import ml_dtypes

import concourse.bass as bass
import concourse.bacc as bacc
import concourse.tile as tile
from concourse import mybir
from concourse.bass_utils import run_bass_kernel_spmd
from concourse.masks import make_identity

F32 = mybir.dt.float32
BF16 = mybir.dt.bfloat16
AF = mybir.ActivationFunctionType
OP = mybir.AluOpType

P = 128
B, S, H = 8, 1024, 1024
NH, HD = 16, 64
HF, HG = 4 * H, H // 2
HC = H // P          # 8 feature chunks
FC = HF // P         # 32 ffn chunks
GC = HG // P         # 4 gate chunks
QT = S // 512        # 2 q tiles of 512
EPS = 1e-5

_BUILD_CACHE = {}


def _bcast_ap(param, n_part, n_free):
    """AP reading a [n_free] DRAM tensor broadcast across n_part partitions."""
    ap = param[None, :]
    return bass.AP(tensor=ap.tensor, offset=ap.offset, ap=[[0, n_part], [1, n_free]])


def _build(flags):
    f = dict(flags)
    nc = bacc.Bacc(None, target_bir_lowering=False)

    dp = nc.declare_dram_parameter
    x_in = dp("x", [S, H], F32, isOutput=False)
    vol = dp("vol", [S], F32, isOutput=False)
    wq = dp("wq", [H, H], BF16, isOutput=False)
    wk = dp("wk", [H, H], BF16, isOutput=False)
    wv = dp("wv", [H, H], BF16, isOutput=False)
    wo = dp("wo", [H, H], BF16, isOutput=False)
    w1 = dp("w1", [H, HF], BF16, isOutput=False)
    w2 = dp("w2", [HF, H], BF16, isOutput=False)
    g1 = dp("g1", [H, HG], BF16, isOutput=False)
    g2 = dp("g2", [HG, H], BF16, isOutput=False)
    bq = dp("bq", [H], F32, isOutput=False)
    bk = dp("bk", [H], F32, isOutput=False)
    bv = dp("bv", [H], F32, isOutput=False)
    bo = dp("bo", [H], F32, isOutput=False)
    b1 = dp("b1", [HF], F32, isOutput=False)
    b2 = dp("b2", [H], F32, isOutput=False)
    gb1 = dp("gb1", [HG], F32, isOutput=False)
    gb2 = dp("gb2", [H], F32, isOutput=False)
    ln1w = dp("ln1w", [H], F32, isOutput=False)
    ln1b = dp("ln1b", [H], F32, isOutput=False)
    ln2w = dp("ln2w", [H], F32, isOutput=False)
    ln2b = dp("ln2b", [H], F32, isOutput=False)
    sc = {}
    for name in ("gamma1", "beta1", "vs1w", "vs1b", "gamma2", "beta2", "vs2w", "vs2b"):
        sc[name] = dp(name, [1], F32, isOutput=False)
    out = dp("out", [S, H], F32, isOutput=True)

    def chunked(param):  # [n] f32 -> [P, n//P] per-partition layout
        return param.rearrange("(c p) -> p c", p=P)

    def wT3(param):  # [K, M] -> [P, K//P, M]
        return param.rearrange("(kc p) m -> p kc m", p=P)

    with tile.TileContext(nc) as tc:
        from contextlib import ExitStack
        with ExitStack() as ctx:
            const = ctx.enter_context(tc.tile_pool(name="const", bufs=1))

            identf = const.tile([P, P], F32)
            make_identity(nc, identf)
            ones_col = const.tile([P, 1], BF16)
            nc.vector.memset(ones_col, 1.0)
            ones_row = const.tile([1, P], BF16)
            nc.vector.memset(ones_row, 1.0)
            eps128 = const.tile([P, 1], F32)
            nc.vector.memset(eps128, EPS)

            # persistent slabs, tag-shared across phases
            trunk = ctx.enter_context(tc.tile_pool(name="trunk", bufs=1))
            xTf = trunk.tile([P, HC, S], F32, tag="f4a", name="xTf")  # x^T -> x1 -> y1
            QTs = trunk.tile([P, HC, S], BF16, tag="bf2e", name="QTs")
            KTs = trunk.tile([P, HC, S], BF16, tag="bf2b", name="KTs")
            Vp = trunk.tile([P, HC, NH, HD + 1], BF16, tag="bf2c", name="Vp")
            xT_bf = trunk.tile([P, HC, S], BF16, tag="bf2d", name="xT_bf")

            # ---------------- P1: PE transpose of x ----------------
            with tc.tile_pool(name="p1", bufs=3) as p1, \
                 tc.tile_pool(name="p1ps", bufs=4, space="PSUM") as p1ps:
                identp = p1.tile([P, P], F32, name="identp", bufs=1)
                make_identity(nc, identp)
                for qc in range(HC):
                    xrow = p1.tile([P, H], F32, tag="xrow")
                    nc.sync.dma_start(out=xrow, in_=x_in[qc * P:(qc + 1) * P, :])
                    for hc in range(HC):
                        pst = p1ps.tile([P, P], F32, tag="ps_tr")
                        nc.tensor.transpose(pst, xrow[:, hc * P:(hc + 1) * P], identp)
                        nc.scalar.activation(xT_bf[:, hc, qc * P:(qc + 1) * P], pst,
                                             AF.Identity)
                        nc.vector.tensor_copy(xTf[:, hc, qc * P:(qc + 1) * P], pst)

            def load_chunked(param, n):
                t = const.tile([P, n], F32, name=f"c_{param.name}")
                nc.sync.dma_start(out=t, in_=chunked(param))
                return t

            bq_sb = load_chunked(bq, HC)
            bk_sb = load_chunked(bk, HC)
            bo_sb = load_chunked(bo, HC) if f["bo"] else None
            b1_sb = load_chunked(b1, FC)
            b2_sb = load_chunked(b2, HC) if f["b2"] else None
            gb1_sb = load_chunked(gb1, GC)
            gb2_sb = load_chunked(gb2, HC)
            if f["bv"]:
                bv_bc = const.tile([P, H], F32)
                nc.gpsimd.dma_start(out=bv_bc, in_=_bcast_ap(bv, P, H))
            if f["ln1w"]:
                ln1w_sb = load_chunked(ln1w, HC)
            if f["ln1b"]:
                ln1b_sb = load_chunked(ln1b, HC)
            if f["ln2w"]:
                ln2w_bc = const.tile([P, H], F32)
                nc.gpsimd.dma_start(out=ln2w_bc, in_=_bcast_ap(ln2w, P, H))
            if f["ln2b"]:
                ln2b_bc = const.tile([P, H], F32)
                nc.gpsimd.dma_start(out=ln2b_bc, in_=_bcast_ap(ln2b, P, H))

            sct = {}
            for name in ("gamma1", "vs1w", "vs1b"):
                t = const.tile([1, 1], F32, name=f"sc_{name}")
                nc.sync.dma_start(out=t, in_=sc[name][None, :])
                sct[name] = t
            for name in ("gamma2", "beta2", "vs2w", "vs2b", "beta1"):
                t = const.tile([P, 1], F32, name=f"sc_{name}")
                nc.gpsimd.dma_start(out=t, in_=_bcast_ap(sc[name], P, 1))
                sct[name] = t

            # volatility-derived per-token scales
            vol_row = const.tile([1, S], F32)
            nc.sync.dma_start(out=vol_row, in_=vol[None, :])
            s1row = const.tile([1, S], F32)
            nc.scalar.activation(s1row, vol_row, AF.Sigmoid,
                                 bias=sct["vs1b"][0:1, :], scale=sct["vs1w"][0:1, :])
            nc.vector.tensor_scalar(s1row, s1row, 1.0, sct["gamma1"],
                                    op0=OP.add, op1=OP.mult)
            vol_np = const.tile([P, HC], F32)
            nc.sync.dma_start(out=vol_np, in_=chunked(vol))
            s2_np = const.tile([P, HC], F32)
            nc.scalar.activation(s2_np, vol_np, AF.Sigmoid,
                                 bias=sct["vs2b"], scale=sct["vs2w"])
            nc.vector.tensor_scalar(s2_np, s2_np, 1.0, sct["gamma2"],
                                    op0=OP.add, op1=OP.mult)

            # ---------------- P2: QKV projections ----------------
            nc.vector.memset(Vp[:, :, :, HD:HD + 1], 1.0)
            with tc.tile_pool(name="p2w", bufs=3) as p2w, \
                 tc.tile_pool(name="p2wv", bufs=2) as p2wv, \
                 tc.tile_pool(name="p2ps", bufs=2, space="PSUM") as p2ps:
                for w_par, dst, bias_sb in ((wq, QTs, bq_sb), (wk, KTs, bk_sb)):
                    for mc in range(HC):
                        wt = p2w.tile([P, HC, P], BF16, tag="wproj")
                        nc.sync.dma_start(out=wt, in_=wT3(w_par)[:, :, mc * P:(mc + 1) * P])
                        for qt in range(QT):
                            ps = p2ps.tile([P, 512], F32, tag="ps_qk")
                            for hc in range(HC):
                                nc.tensor.matmul(ps, wt[:, hc, :],
                                                 xT_bf[:, hc, qt * 512:(qt + 1) * 512],
                                                 start=(hc == 0), stop=(hc == HC - 1))
                            nc.scalar.activation(dst[:, mc, qt * 512:(qt + 1) * 512], ps,
                                                 AF.Identity, bias=bias_sb[:, mc:mc + 1])
                for dt in range(2):
                    wt = p2wv.tile([P, HC, 512], BF16, tag="wv")
                    nc.sync.dma_start(out=wt, in_=wT3(wv)[:, :, dt * 512:(dt + 1) * 512])
                    for kc in range(HC):
                        ps = p2ps.tile([P, 512], F32, tag="ps_v")
                        for hc in range(HC):
                            nc.tensor.matmul(ps, xT_bf[:, hc, kc * P:(kc + 1) * P],
                                             wt[:, hc, :],
                                             start=(hc == 0), stop=(hc == HC - 1))
                        dst = Vp[:, kc, dt * 8:(dt + 1) * 8, 0:HD]
                        src = ps.rearrange("p (h d) -> p h d", d=HD)
                        if f["bv"]:
                            nc.vector.tensor_tensor(
                                dst, src,
                                bv_bc[:, dt * 512:(dt + 1) * 512].rearrange(
                                    "p (h d) -> p h d", d=HD), OP.add)
                        else:
                            nc.scalar.activation(dst, src, AF.Identity)

            # ---------------- P3: attention ----------------
            # xT_bf dead; its tag slot becomes ctxT
            ctxT = trunk.tile([P, HC, S], BF16, tag="bf2d", name="ctxT")
            with tc.tile_pool(name="p3e", bufs=2) as p3e, \
                 tc.tile_pool(name="p3r", bufs=2) as p3r, \
                 tc.tile_pool(name="p3ps", bufs=2, space="PSUM") as p3ps:
                for hp in range(NH // 2):
                    pair = [(2 * hp, p3e.tile([P, HC, S], BF16, tag="E", name="e0")),
                            (2 * hp + 1, p3e.tile([P, HC, S], BF16, tag="E", name="e1"))]
                    for kc in range(HC):
                        for h, e in pair:
                            p0 = 64 * (h % 2)
                            hcx = h // 2
                            ps = p3ps.tile([P, S], F32, tag="ps_s")
                            for qt in range(QT):
                                nc.tensor.matmul(
                                    ps[:, qt * 512:(qt + 1) * 512],
                                    KTs[p0:p0 + 64, hcx, kc * P:(kc + 1) * P],
                                    QTs[p0:p0 + 64, hcx, qt * 512:(qt + 1) * 512],
                                    start=True, stop=True)
                            nc.scalar.activation(e[:, kc, :], ps, AF.Exp, scale=0.125)
                    for h, e in pair:
                        p0 = 64 * (h % 2)
                        for qt in range(QT):
                            pav = p3ps.tile([65, 512], F32, tag="ps_av")
                            for kc in range(HC):
                                nc.tensor.matmul(pav, Vp[:, kc, h, :],
                                                 e[:, kc, qt * 512:(qt + 1) * 512],
                                                 start=(kc == 0), stop=(kc == HC - 1))
                            # broadcast the raw rowsum with a K=1 matmul, then
                            # reciprocal across all 64 lanes in one approx op
                            rs = p3r.tile([1, 512], BF16, tag="rsum")
                            nc.vector.tensor_copy(rs, pav[64:65, :])
                            pbc = p3ps.tile([64, 512], F32, tag="ps_bc")
                            nc.tensor.matmul(pbc, ones_row[0:1, 0:64], rs,
                                             start=True, stop=True)
                            rec = p3r.tile([64, 512], F32, tag="rec")
                            nc.vector.reciprocal_approx_fast(out=rec, in_=pbc)
                            nc.vector.tensor_tensor(
                                ctxT[p0:p0 + 64, h // 2, qt * 512:(qt + 1) * 512],
                                rec, pav[0:64, :], OP.mult)

            # ---------------- P4+P5: Wo + residual + LN1 + gate ----------------
            # single psum pool across Wo/LN1/gate so gate matmuls can fill
            # the LN1 dependency chain; stats share one bank (sum@row0,
            # sumsq@row32)
            x1bf = trunk.tile([P, HC, S], BF16, tag="bf2a", name="x1bf")
            gT = trunk.tile([P, HC, S], BF16, tag="bf2e", name="gT")
            rT = trunk.tile([P, GC, S], BF16, tag="bf2c", name="rT")
            g1_bufs = 1 if f["ln1b"] else 2
            with tc.tile_pool(name="pw", bufs=3) as pw:
              with tc.tile_pool(name="pt4", bufs=1) as pt4, \
                   tc.tile_pool(name="pAps", bufs=1, space="PSUM") as pAps:
                for qt in range(QT):
                    sl = slice(qt * 512, (qt + 1) * 512)
                    for mc in range(HC):
                        wt = pw.tile([P, HC, P], BF16, tag="wproj", bufs=3)
                        nc.sync.dma_start(out=wt, in_=wT3(wo)[:, :, mc * P:(mc + 1) * P])
                        ps = pAps.tile([P, 512], F32, tag="ps_o", bufs=2)
                        for dc in range(HC):
                            nc.tensor.matmul(ps, wt[:, dc, :],
                                             ctxT[:, dc, qt * 512:(qt + 1) * 512],
                                             start=(dc == 0), stop=(dc == HC - 1))
                        xs = xTf[:, mc, sl]
                        nc.vector.tensor_tensor(xs, ps, xs, OP.add)
                        if f["bo"]:
                            nc.vector.tensor_scalar(xs, xs, bo_sb[:, mc:mc + 1], None,
                                                    op0=OP.add)
                        nc.scalar.activation(x1bf[:, mc, sl], xs, AF.Identity)
                    # LN1 for this q-tile; xTf: x1 -> y1 in place
                    pstat = pAps.tile([33, 512], F32, tag="ps_stat")
                    for mc in range(HC):
                        nc.tensor.matmul(pstat[0:1, :], ones_col, x1bf[:, mc, sl],
                                         start=(mc == 0), stop=(mc == HC - 1))
                    sq = pt4.tile([P, HC, 512], BF16, tag="sq")
                    nc.scalar.activation(sq, xTf[:, :, sl], AF.Square)
                    for mc in range(HC):
                        nc.tensor.matmul(pstat[32:33, :], ones_col, sq[:, mc, :],
                                         start=(mc == 0), stop=(mc == HC - 1))
                    mu = pt4.tile([1, 512], F32, tag="mu")
                    nc.vector.tensor_scalar(mu, pstat[0:1, :], 1.0 / H, None, op0=OP.mult)
                    mu2 = pt4.tile([1, 512], F32, tag="mu2")
                    nc.vector.tensor_tensor(mu2, mu, mu, OP.mult)
                    var = pt4.tile([1, 512], F32, tag="var")
                    # var = sumsq/H - mu^2 in one op
                    nc.vector.scalar_tensor_tensor(var, pstat[32:33, :], 1.0 / H, mu2,
                                                   op0=OP.mult, op1=OP.subtract)
                    nc.scalar.activation(var, var, AF.Sqrt, bias=eps128[0:1, :])
                    rstd = pt4.tile([1, 512], F32, tag="rstd")
                    nc.vector.reciprocal_approx_fast(out=rstd, in_=var)
                    arow = pt4.tile([1, 512], F32, tag="arow")
                    nc.vector.tensor_tensor(arow, rstd, s1row[0:1, sl], OP.mult)
                    arow_bf = pt4.tile([1, 512], BF16, tag="arow_bf")
                    nc.vector.tensor_copy(arow_bf, arow)
                    crow_bf = pt4.tile([1, 512], BF16, tag="crow_bf")
                    nc.vector.tensor_tensor(crow_bf, mu, arow, OP.mult)
                    psa = pAps.tile([P, 512], F32, tag="ps_a")
                    nc.tensor.matmul(psa, ones_row, arow_bf, start=True, stop=True)
                    psc = pAps.tile([P, 512], F32, tag="ps_c")
                    nc.tensor.matmul(psc, ones_row, crow_bf, start=True, stop=True)
                    if f["ln1b"]:
                        s1_bf = pt4.tile([1, 512], BF16, tag="s1_bf")
                        nc.vector.tensor_copy(s1_bf, s1row[0:1, sl])
                        pss1 = pAps.tile([P, 512], F32, tag="ps_s1")
                        nc.tensor.matmul(pss1, ones_row, s1_bf, start=True, stop=True)
                    for mc in range(HC):
                        y = xTf[:, mc, sl]
                        nc.vector.tensor_tensor(y, y, psa, OP.mult)
                        nc.vector.tensor_tensor(y, y, psc, OP.subtract)
                        if f["ln1w"]:
                            nc.vector.tensor_scalar(y, y, ln1w_sb[:, mc:mc + 1], None,
                                                    op0=OP.mult)
                        if f["ln1b"]:
                            bs = pt4.tile([P, 512], F32, tag="bs")
                            nc.vector.tensor_scalar(bs, pss1, ln1b_sb[:, mc:mc + 1],
                                                    None, op0=OP.mult)
                            nc.vector.tensor_tensor(y, y, bs, OP.add)
                        if f["beta1"]:
                            nc.vector.tensor_scalar(y, y, sct["beta1"], None, op0=OP.add)
                        nc.scalar.activation(x1bf[:, mc, sl], y, AF.Identity)

                # gate first layer (runs while LN1 of the second q-tile drains)
                for qt in range(QT):
                    sl = slice(qt * 512, (qt + 1) * 512)
                    for mc in range(GC):
                        wt = pw.tile([P, HC, P], BF16, tag="wproj", bufs=3)
                        nc.sync.dma_start(out=wt, in_=wT3(g1)[:, :, mc * P:(mc + 1) * P])
                        ps = pAps.tile([P, 512], F32, tag="ps_g1", bufs=g1_bufs)
                        for hc in range(HC):
                            nc.tensor.matmul(ps, wt[:, hc, :], x1bf[:, hc, sl],
                                             start=(hc == 0), stop=(hc == HC - 1))
                        nc.scalar.activation(rT[:, mc, sl], ps, AF.Relu,
                                             bias=gb1_sb[:, mc:mc + 1])

              y1bf = x1bf  # bf16 y1; xTf holds f32 y1

              # ---------------- P6: gate2 + FFN + gated mix; P7 LN2 --------
              # hT for a full ffn-half lives in the dead KTs/ctxT slabs; g2,
              # W1-build, W2-accumulate and P7 transposes all rotate through
              # the same four psum tags (2 banks each) so W1/W2 stream once.
              accf = trunk.tile([P, HC, S], F32, tag="f4c", name="accf")
              if True:
                with tc.tile_pool(name="pt7", bufs=2) as pt7, \
                     tc.tile_pool(name="pCps", bufs=1, space="PSUM") as pCps:
                    psk = [0]

                    def accps(shape):
                        t = pCps.tile(shape, F32, tag=f"ps_acc{psk[0] % 4}",
                                      name=f"psacc{psk[0] % 4}")
                        psk[0] += 1
                        return t

                    for qt in range(QT):
                        for mc in range(HC):
                            wt = pw.tile([P, GC, P], BF16, tag="wg2", bufs=3)
                            nc.sync.dma_start(out=wt,
                                              in_=g2.rearrange("(gc p) o -> p gc o", p=P)
                                              [:, :, mc * P:(mc + 1) * P])
                            ps = accps([P, 512])
                            for gc in range(GC):
                                nc.tensor.matmul(ps, wt[:, gc, :],
                                                 rT[:, gc, qt * 512:(qt + 1) * 512],
                                                 start=(gc == 0), stop=(gc == GC - 1))
                            nc.scalar.activation(gT[:, mc, qt * 512:(qt + 1) * 512], ps,
                                                 AF.Sigmoid, bias=gb2_sb[:, mc:mc + 1])
                    for half in range(2):
                        hA = trunk.tile([P, 8, S], BF16, tag="bf2b", name="hA")
                        hB = trunk.tile([P, 8, S], BF16, tag="bf2d", name="hB")

                        def hsl(c, qsl=slice(None)):
                            return (hA[:, c, qsl] if c < 8 else hB[:, c - 8, qsl])

                        for c in range(16):
                            cg = half * 16 + c
                            wt = pw.tile([P, HC, P], BF16, tag="wproj", bufs=3)
                            nc.sync.dma_start(out=wt,
                                              in_=wT3(w1)[:, :, cg * P:(cg + 1) * P])
                            psh = accps([P, S])
                            for qt in range(QT):
                                for hc in range(HC):
                                    nc.tensor.matmul(
                                        psh[:, qt * 512:(qt + 1) * 512], wt[:, hc, :],
                                        y1bf[:, hc, qt * 512:(qt + 1) * 512],
                                        start=(hc == 0), stop=(hc == HC - 1))
                            nc.scalar.activation(hsl(c), psh, AF.Gelu,
                                                 bias=b1_sb[:, cg:cg + 1])
                        for oh in range(2):
                            accs = [accps([P, S]) for mc in range(4)]
                            for c in range(16):
                                cg = half * 16 + c
                                wt = pw.tile([P, 512], BF16, tag="w2", bufs=6)
                                nc.sync.dma_start(
                                    out=wt,
                                    in_=w2.rearrange("(cc p) o -> p cc o", p=P)
                                    [:, cg, oh * 512:(oh + 1) * 512])
                                for mc in range(4):
                                    for qt in range(QT):
                                        nc.tensor.matmul(
                                            accs[mc][:, qt * 512:(qt + 1) * 512],
                                            wt[:, mc * P:(mc + 1) * P],
                                            hsl(c, slice(qt * 512, (qt + 1) * 512)),
                                            start=(c == 0), stop=(c == 15))
                            for mc in range(4):
                                mcg = oh * 4 + mc
                                for qt in range(QT):
                                    a = accf[:, mcg, qt * 512:(qt + 1) * 512]
                                    psl = accs[mc][:, qt * 512:(qt + 1) * 512]
                                    y = xTf[:, mcg, qt * 512:(qt + 1) * 512]
                                    if half == 0:
                                        # store (ffn_half0 - y1): saves a pass later
                                        nc.vector.tensor_tensor(a, psl, y, OP.subtract)
                                    else:
                                        nc.vector.tensor_tensor(a, a, psl, OP.add)
                                        if f["b2"]:
                                            nc.vector.tensor_scalar(
                                                a, a, b2_sb[:, mcg:mcg + 1], None,
                                                op0=OP.add)
                                        g = gT[:, mcg, qt * 512:(qt + 1) * 512]
                                        nc.vector.tensor_tensor(a, a, g, OP.mult)
                                        nc.vector.scalar_tensor_tensor(
                                            a, y, 2.0, a, op0=OP.mult, op1=OP.add)

                    # ---------------- P7: LN2 (N-layout) + output ------------
                    for qc in range(HC):
                        xt = pt7.tile([P, H], F32, tag="x2")
                        for hc in range(HC):
                            pst = accps([P, P])
                            nc.tensor.transpose(pst, accf[:, hc, qc * P:(qc + 1) * P],
                                                identf)
                            nc.scalar.activation(xt[:, hc * P:(hc + 1) * P], pst,
                                                 AF.Identity)
                        stats = pt7.tile([P, 2, nc.vector.BN_STATS_DIM], F32, tag="stats")
                        for sg in range(2):
                            nc.vector.bn_stats(stats[:, sg, :],
                                               xt[:, sg * 512:(sg + 1) * 512])
                        mv = pt7.tile([P, nc.vector.BN_AGGR_DIM], F32, tag="mv")
                        nc.vector.bn_aggr(mv, stats)
                        sd = pt7.tile([P, 1], F32, tag="sd")
                        nc.scalar.activation(sd, mv[:, 1:2], AF.Sqrt, bias=eps128)
                        rstd2 = pt7.tile([P, 1], F32, tag="rstd2")
                        nc.vector.reciprocal(rstd2, sd)
                        a2 = pt7.tile([P, 1], F32, tag="a2")
                        nc.vector.tensor_tensor(a2, rstd2, s2_np[:, qc:qc + 1], OP.mult)
                        ot = pt7.tile([P, H], F32, tag="ot")
                        nc.vector.tensor_scalar(ot, xt, mv[:, 0:1], a2,
                                                op0=OP.subtract, op1=OP.mult)
                        if f["ln2w"]:
                            nc.vector.tensor_tensor(ot, ot, ln2w_bc, OP.mult)
                        if f["ln2b"]:
                            bs2 = pt7.tile([P, H], F32, tag="bs2")
                            nc.vector.tensor_scalar(bs2, ln2b_bc, s2_np[:, qc:qc + 1],
                                                    None, op0=OP.mult)
                            nc.vector.tensor_tensor(ot, ot, bs2, OP.add)
                        if f["beta2"]:
                            nc.vector.tensor_scalar(ot, ot, sct["beta2"], None,
                                                    op0=OP.add)
                        nc.sync.dma_start(out=out[qc * P:(qc + 1) * P, :], in_=ot)

    nc.compile()
    return nc


def _prep(inputs):
    """Host-side prep: per-core in_maps (DP over batch) + build flags."""
    bf = ml_dtypes.bfloat16
    x = np.asarray(inputs["x"], np.float32)
    volat = np.asarray(inputs["volatility"], np.float32)

    def w_bf(name):
        return np.ascontiguousarray(np.asarray(inputs[name], np.float32).astype(bf))

    shared = {
        "wq": w_bf("Wq"), "wk": w_bf("Wk"), "wv": w_bf("Wv"), "wo": w_bf("Wo"),
        "w1": w_bf("ffn_w1"), "w2": w_bf("ffn_w2"),
        "g1": w_bf("gate_w1"), "g2": w_bf("gate_w2"),
    }
    for name, key in (("bq", "bq"), ("bk", "bk"), ("bv", "bv"), ("bo", "bo"),
                      ("b1", "ffn_b1"), ("b2", "ffn_b2"),
                      ("gb1", "gate_b1"), ("gb2", "gate_b2"),
                      ("ln1w", "ln1_w"), ("ln1b", "ln1_b"),
                      ("ln2w", "ln2_w"), ("ln2b", "ln2_b")):
        shared[name] = np.ascontiguousarray(np.asarray(inputs[key], np.float32))
    for name, key in (("gamma1", "gamma1"), ("beta1", "beta1"),
                      ("vs1w", "vs1_w"), ("vs1b", "vs1_b"),
                      ("gamma2", "gamma2"), ("beta2", "beta2"),
                      ("vs2w", "vs2_w"), ("vs2b", "vs2_b")):
        shared[name] = np.asarray(inputs[key], np.float32).reshape(1)

    flags = (
        ("bv", bool(np.any(shared["bv"]))),
        ("bo", bool(np.any(shared["bo"]))),
        ("b2", bool(np.any(shared["b2"]))),
        ("ln1w", bool(np.any(shared["ln1w"] != 1.0))),
        ("ln1b", bool(np.any(shared["ln1b"]))),
        ("beta1", bool(shared["beta1"][0] != 0.0)),
        ("ln2w", bool(np.any(shared["ln2w"] != 1.0))),
        ("ln2b", bool(np.any(shared["ln2b"]))),
        ("beta2", bool(shared["beta2"][0] != 0.0)),
    )

    in_maps = []
    for b in range(B):
        m = dict(shared)
        m["x"] = np.ascontiguousarray(x[b])
        m["vol"] = np.ascontiguousarray(volat[b])
        in_maps.append(m)
    return in_maps, flags


def _run(inputs, trace=False):
    in_maps, flags = _prep(inputs)
    if flags not in _BUILD_CACHE:
        _BUILD_CACHE[flags] = _build(flags)
    nc = _BUILD_CACHE[flags]
    res = run_bass_kernel_spmd(nc, in_maps, core_ids=list(range(B)), trace=trace)
    outs = np.stack([res.results[b]["out"] for b in range(B)], axis=0)
    return outs.astype(np.float32), res


def kernel(**inputs) -> np.ndarray:
    out, _ = _run(inputs, trace=False)
    return out



# revision 2
# speedup vs baseline: 1.2072x; 1.2072x over previous
"""EnhancedTransformerBlock on 8 TRN2 NeuronCores.

Strategy: pure data-parallel over batch (B=8 -> 1 element/core, no
collectives). Per core the block runs in "T-layout" ([feature, token],
features on partitions) so every matmul contracts over the partition dim.

R1 changes vs baseline:
- x is transposed on the HOST (numpy) and shipped both as fp8 (matmul
  path) and f32 (residual path); the on-device PE transpose phase is gone.
- All weight-stationary matmuls except the attention score matmuls run as
  fp8e4 DoubleRow (2 k-chunks per matmul, ~2x stream rate): QKV, AV, Wo,
  gate1/2, ffn w1/w2. Weights are pre-scaled by 128 on host (fp8 subnormal
  avoidance) and unscaled for free via the eviction's scale= operand.
- ctx is carried at 16x scale (fp8 subnormal avoidance), unscaled in the
  Wo eviction.
- Q/K/V psum evictions moved from ScalarE to VectorE (ScalarE is the
  attention-phase bottleneck: 16.8M exp elements).

Softmax: scores^T = K_h Q_h^T per head in bf16 (no max subtraction needed
-- the logits are O(1)), exp on ACT straight out of PSUM (fp8 out), row
sums via an appended ones-column on V (row 64 of the AV matmul output),
normalization by a K=1 "broadcast" matmul of the reciprocal row. Heads are
processed in pairs mapped to PE row-groups 0-1/2-3 so their K=64 score
matmuls overlap.

LayerNorm1 in T-layout: column sums via ones-column matmuls, per-token
scale/shift broadcast across partitions with K=1 matmuls. LayerNorm2 in
N-layout after PE f32 transposes (bn_stats/bn_aggr), writing the output
directly in [token, feature] order.

SBUF slabs are tag-shared across phases: same tag + bufs=1 reuses the
address, Tile's WAR tracking orders the phases.
"""
import sys

sys.path.insert(0, '/opt/trn_rl_repo')

import numpy as np
import ml_dtypes

import concourse.bass as bass
import concourse.bacc as bacc
import concourse.tile as tile
from concourse import mybir
from concourse.bass_utils import run_bass_kernel_spmd
from concourse.masks import make_identity

F32 = mybir.dt.float32
BF16 = mybir.dt.bfloat16
F8 = mybir.dt.float8e4
AF = mybir.ActivationFunctionType
OP = mybir.AluOpType
DR = mybir.MatmulPerfMode.DoubleRow

P = 128
B, S, H = 8, 1024, 1024
NH, HD = 16, 64
HF, HG = 4 * H, H // 2
HC = H // P          # 8 feature chunks
FC = HF // P         # 32 ffn chunks
GC = HG // P         # 4 gate chunks
QT = S // 512        # 2 q tiles of 512
EPS = 1e-5
WSC = 128.0          # host-side weight scale (fp8 subnormal avoidance)
WS = 1.0 / WSC
CTXS = 16.0          # ctx carried at 16x in fp8

_BUILD_CACHE = {}


def _bcast_ap(param, n_part, n_free):
    """AP reading a [n_free] DRAM tensor broadcast across n_part partitions."""
    ap = param[None, :]
    return bass.AP(tensor=ap.tensor, offset=ap.offset, ap=[[0, n_part], [1, n_free]])


def _build(flags):
    f = dict(flags)
    nc = bacc.Bacc(None, target_bir_lowering=False)

    dp = nc.declare_dram_parameter
    xt8 = dp("xt8", [H, S], F8, isOutput=False)
    xtf = dp("xtf", [H, S], F32, isOutput=False)
    vol = dp("vol", [S], F32, isOutput=False)
    wq = dp("wq", [H, H], F8, isOutput=False)
    wk = dp("wk", [H, H], F8, isOutput=False)
    wv = dp("wv", [H, H], F8, isOutput=False)
    wo = dp("wo", [H, H], F8, isOutput=False)
    w1 = dp("w1", [H, HF], F8, isOutput=False)
    w2 = dp("w2", [HF, H], F8, isOutput=False)
    g1 = dp("g1", [H, HG], F8, isOutput=False)
    g2 = dp("g2", [HG, H], F8, isOutput=False)
    bq = dp("bq", [H], F32, isOutput=False)
    bk = dp("bk", [H], F32, isOutput=False)
    bv = dp("bv", [H], F32, isOutput=False)
    bo = dp("bo", [H], F32, isOutput=False)
    b1 = dp("b1", [HF], F32, isOutput=False)
    b2 = dp("b2", [H], F32, isOutput=False)
    gb1 = dp("gb1", [HG], F32, isOutput=False)
    gb2 = dp("gb2", [H], F32, isOutput=False)
    ln1w = dp("ln1w", [H], F32, isOutput=False)
    ln1b = dp("ln1b", [H], F32, isOutput=False)
    ln2w = dp("ln2w", [H], F32, isOutput=False)
    ln2b = dp("ln2b", [H], F32, isOutput=False)
    sc = {}
    for name in ("gamma1", "beta1", "vs1w", "vs1b", "gamma2", "beta2", "vs2w", "vs2b"):
        sc[name] = dp(name, [1], F32, isOutput=False)
    out = dp("out", [S, H], F32, isOutput=True)

    def chunked(param):  # [n] f32 -> [P, n//P] per-partition layout
        return param.rearrange("(c p) -> p c", p=P)

    def wT3(param):  # [K, M] -> [P, K//P, M]
        return param.rearrange("(kc p) m -> p kc m", p=P)

    with tile.TileContext(nc) as tc:
        from contextlib import ExitStack
        with ExitStack() as ctx:
            const = ctx.enter_context(tc.tile_pool(name="const", bufs=1))

            identf = const.tile([P, P], F32)
            make_identity(nc, identf)
            ones_col = const.tile([P, 1], BF16)
            nc.vector.memset(ones_col, 1.0)
            ones_f8 = const.tile([P, 1], F8)
            nc.vector.memset(ones_f8, 1.0)
            ones_row = const.tile([1, P], BF16)
            nc.vector.memset(ones_row, 1.0)
            eps128 = const.tile([P, 1], F32)
            nc.vector.memset(eps128, EPS)

            # persistent slabs, tag-shared across phases
            trunk = ctx.enter_context(tc.tile_pool(name="trunk", bufs=1))
            xTf = trunk.tile([P, HC, S], F32, tag="f4a", name="xTf")  # x^T -> x1 -> y1
            QTs = trunk.tile([P, HC, S], BF16, tag="bf2e", name="QTs")
            KTs = trunk.tile([P, HC, S], BF16, tag="bf2b", name="KTs")
            Vp = trunk.tile([P, HC, NH, HD + 1], F8, tag="bf2c", name="Vp")
            xT8 = trunk.tile([P, HC, S], F8, tag="bf2d", name="xT8")

            # host-pretransposed x: fp8 (matmul) + f32 (residual)
            nc.sync.dma_start(out=xT8, in_=xt8.rearrange("(c p) s -> p c s", p=P))
            nc.sync.dma_start(out=xTf, in_=xtf.rearrange("(c p) s -> p c s", p=P))

            def load_chunked(param, n):
                t = const.tile([P, n], F32, name=f"c_{param.name}")
                nc.sync.dma_start(out=t, in_=chunked(param))
                return t

            bq_sb = load_chunked(bq, HC)
            bk_sb = load_chunked(bk, HC)
            bo_sb = load_chunked(bo, HC) if f["bo"] else None
            b1_sb = load_chunked(b1, FC)
            b2_sb = load_chunked(b2, HC) if f["b2"] else None
            gb1_sb = load_chunked(gb1, GC)
            gb2_sb = load_chunked(gb2, HC)
            if f["bv"]:
                bv_bc = const.tile([P, H], F32)
                nc.gpsimd.dma_start(out=bv_bc, in_=_bcast_ap(bv, P, H))
            if f["ln1w"]:
                ln1w_sb = load_chunked(ln1w, HC)
            if f["ln1b"]:
                ln1b_sb = load_chunked(ln1b, HC)
            if f["ln2w"]:
                ln2w_bc = const.tile([P, H], F32)
                nc.gpsimd.dma_start(out=ln2w_bc, in_=_bcast_ap(ln2w, P, H))
            if f["ln2b"]:
                ln2b_bc = const.tile([P, H], F32)
                nc.gpsimd.dma_start(out=ln2b_bc, in_=_bcast_ap(ln2b, P, H))

            sct = {}
            for name in ("gamma1", "vs1w", "vs1b"):
                t = const.tile([1, 1], F32, name=f"sc_{name}")
                nc.sync.dma_start(out=t, in_=sc[name][None, :])
                sct[name] = t
            for name in ("gamma2", "beta2", "vs2w", "vs2b", "beta1"):
                t = const.tile([P, 1], F32, name=f"sc_{name}")
                nc.gpsimd.dma_start(out=t, in_=_bcast_ap(sc[name], P, 1))
                sct[name] = t

            # volatility-derived per-token scales
            vol_row = const.tile([1, S], F32)
            nc.sync.dma_start(out=vol_row, in_=vol[None, :])
            s1row = const.tile([1, S], F32)
            nc.scalar.activation(s1row, vol_row, AF.Sigmoid,
                                 bias=sct["vs1b"][0:1, :], scale=sct["vs1w"][0:1, :])
            nc.vector.tensor_scalar(s1row, s1row, 1.0, sct["gamma1"],
                                    op0=OP.add, op1=OP.mult)
            vol_np = const.tile([P, HC], F32)
            nc.sync.dma_start(out=vol_np, in_=chunked(vol))
            s2_np = const.tile([P, HC], F32)
            nc.scalar.activation(s2_np, vol_np, AF.Sigmoid,
                                 bias=sct["vs2b"], scale=sct["vs2w"])
            nc.vector.tensor_scalar(s2_np, s2_np, 1.0, sct["gamma2"],
                                    op0=OP.add, op1=OP.mult)

            # ---------------- P2: QKV projections (fp8 DoubleRow) ----------
            nc.vector.memset(Vp[:, :, :, HD:HD + 1], 1.0)
            with tc.tile_pool(name="p2w", bufs=3) as p2w, \
                 tc.tile_pool(name="p2wv", bufs=2) as p2wv, \
                 tc.tile_pool(name="p2ps", bufs=2, space="PSUM") as p2ps:
                for w_par, dst, bias_sb in ((wq, QTs, bq_sb), (wk, KTs, bk_sb)):
                    for mc in range(HC):
                        wt = p2w.tile([P, HC, P], F8, tag="wproj")
                        nc.sync.dma_start(out=wt, in_=wT3(w_par)[:, :, mc * P:(mc + 1) * P])
                        for qt in range(QT):
                            ps = p2ps.tile([P, 512], F32, tag="ps_qk")
                            for g in range(HC // 2):
                                nc.tensor.matmul(ps, wt[:, 2 * g:2 * g + 2, :],
                                                 xT8[:, 2 * g:2 * g + 2,
                                                     qt * 512:(qt + 1) * 512],
                                                 perf_mode=DR,
                                                 start=(g == 0), stop=(g == HC // 2 - 1))
                            nc.vector.tensor_scalar(
                                dst[:, mc, qt * 512:(qt + 1) * 512], ps, WS,
                                bias_sb[:, mc:mc + 1], op0=OP.mult, op1=OP.add)
                for dt in range(2):
                    wt = p2wv.tile([P, HC, 512], F8, tag="wv")
                    nc.sync.dma_start(out=wt, in_=wT3(wv)[:, :, dt * 512:(dt + 1) * 512])
                    for kc in range(HC):
                        ps = p2ps.tile([P, 512], F32, tag="ps_v")
                        for g in range(HC // 2):
                            nc.tensor.matmul(ps,
                                             xT8[:, 2 * g:2 * g + 2, kc * P:(kc + 1) * P],
                                             wt[:, 2 * g:2 * g + 2, :],
                                             perf_mode=DR,
                                             start=(g == 0), stop=(g == HC // 2 - 1))
                        dst = Vp[:, kc, dt * 8:(dt + 1) * 8, 0:HD]
                        src = ps.rearrange("p (h d) -> p h d", d=HD)
                        if f["bv"]:
                            nc.vector.scalar_tensor_tensor(
                                dst, src, WS,
                                bv_bc[:, dt * 512:(dt + 1) * 512].rearrange(
                                    "p (h d) -> p h d", d=HD),
                                op0=OP.mult, op1=OP.add)
                        else:
                            nc.vector.tensor_scalar(dst, src, WS, None, op0=OP.mult)

            # ---------------- P3: attention ----------------
            # xT8 dead; its tag slot becomes ctxT (fp8, 16x scale)
            ctxT = trunk.tile([P, HC, S], F8, tag="bf2d", name="ctxT")
            with tc.tile_pool(name="p3e", bufs=2) as p3e, \
                 tc.tile_pool(name="p3r", bufs=2) as p3r, \
                 tc.tile_pool(name="p3ps", bufs=2, space="PSUM") as p3ps:
                for hp in range(NH // 2):
                    pair = [(2 * hp, p3e.tile([P, HC, S], F8, tag="E", name="e0")),
                            (2 * hp + 1, p3e.tile([P, HC, S], F8, tag="E", name="e1"))]
                    for kc in range(HC):
                        for h, e in pair:
                            p0 = 64 * (h % 2)
                            hcx = h // 2
                            ps = p3ps.tile([P, S], F32, tag="ps_s")
                            for qt in range(QT):
                                nc.tensor.matmul(
                                    ps[:, qt * 512:(qt + 1) * 512],
                                    KTs[p0:p0 + 64, hcx, kc * P:(kc + 1) * P],
                                    QTs[p0:p0 + 64, hcx, qt * 512:(qt + 1) * 512],
                                    start=True, stop=True)
                            nc.scalar.activation(e[:, kc, :], ps, AF.Exp, scale=0.125)
                    for h, e in pair:
                        p0 = 64 * (h % 2)
                        for qt in range(QT):
                            pav = p3ps.tile([65, 512], F32, tag="ps_av")
                            for g in range(HC // 2):
                                nc.tensor.matmul(pav,
                                                 Vp[:, 2 * g:2 * g + 2, h, :],
                                                 e[:, 2 * g:2 * g + 2,
                                                   qt * 512:(qt + 1) * 512],
                                                 perf_mode=DR,
                                                 start=(g == 0), stop=(g == HC // 2 - 1))
                            # broadcast the raw rowsum with a K=1 matmul, then
                            # reciprocal across all 64 lanes in one approx op
                            rs = p3r.tile([1, 512], BF16, tag="rsum")
                            nc.vector.tensor_scalar(rs, pav[64:65, :], 1.0 / CTXS,
                                                    None, op0=OP.mult)
                            pbc = p3ps.tile([64, 512], F32, tag="ps_bc")
                            nc.tensor.matmul(pbc, ones_row[0:1, 0:64], rs,
                                             start=True, stop=True)
                            rec = p3r.tile([64, 512], F32, tag="rec")
                            nc.vector.reciprocal_approx_fast(out=rec, in_=pbc)
                            nc.vector.tensor_tensor(
                                ctxT[p0:p0 + 64, h // 2, qt * 512:(qt + 1) * 512],
                                rec, pav[0:64, :], OP.mult)

            # ---------------- P4+P5: Wo + residual + LN1 + gate ----------------
            # single psum pool across Wo/LN1/gate so gate matmuls can fill
            # the LN1 dependency chain; stats share one bank (sum@row0,
            # sumsq@row32)
            x1f8 = trunk.tile([P, HC, S], F8, tag="bf2a", name="x1f8")
            gT = trunk.tile([P, HC, S], BF16, tag="bf2e", name="gT")
            rT = trunk.tile([P, GC, S], F8, tag="bf2c", name="rT")
            g1_bufs = 1 if f["ln1b"] else 2
            with tc.tile_pool(name="pw", bufs=3) as pw:
              with tc.tile_pool(name="pt4", bufs=1) as pt4, \
                   tc.tile_pool(name="pAps", bufs=1, space="PSUM") as pAps:
                for qt in range(QT):
                    sl = slice(qt * 512, (qt + 1) * 512)
                    for mc in range(HC):
                        wt = pw.tile([P, HC, P], F8, tag="wproj", bufs=3)
                        nc.sync.dma_start(out=wt, in_=wT3(wo)[:, :, mc * P:(mc + 1) * P])
                        ps = pAps.tile([P, 512], F32, tag="ps_o", bufs=2)
                        for g in range(HC // 2):
                            nc.tensor.matmul(ps, wt[:, 2 * g:2 * g + 2, :],
                                             ctxT[:, 2 * g:2 * g + 2,
                                                  qt * 512:(qt + 1) * 512],
                                             perf_mode=DR,
                                             start=(g == 0), stop=(g == HC // 2 - 1))
                        xs = xTf[:, mc, sl]
                        nc.vector.scalar_tensor_tensor(xs, ps, WS / CTXS, xs,
                                                       op0=OP.mult, op1=OP.add)
                        if f["bo"]:
                            nc.vector.tensor_scalar(xs, xs, bo_sb[:, mc:mc + 1], None,
                                                    op0=OP.add)
                        nc.scalar.activation(x1f8[:, mc, sl], xs, AF.Identity)
                    # LN1 for this q-tile; xTf: x1 -> y1 in place
                    pstat = pAps.tile([33, 512], F32, tag="ps_stat")
                    for mc in range(HC):
                        nc.tensor.matmul(pstat[0:1, :], ones_f8, x1f8[:, mc, sl],
                                         start=(mc == 0), stop=(mc == HC - 1))
                    sq = pt4.tile([P, HC, 512], BF16, tag="sq")
                    nc.scalar.activation(sq, xTf[:, :, sl], AF.Square)
                    for mc in range(HC):
                        nc.tensor.matmul(pstat[32:33, :], ones_col, sq[:, mc, :],
                                         start=(mc == 0), stop=(mc == HC - 1))
                    mu = pt4.tile([1, 512], F32, tag="mu")
                    nc.vector.tensor_scalar(mu, pstat[0:1, :], 1.0 / H, None, op0=OP.mult)
                    mu2 = pt4.tile([1, 512], F32, tag="mu2")
                    nc.vector.tensor_tensor(mu2, mu, mu, OP.mult)
                    var = pt4.tile([1, 512], F32, tag="var")
                    # var = sumsq/H - mu^2 in one op
                    nc.vector.scalar_tensor_tensor(var, pstat[32:33, :], 1.0 / H, mu2,
                                                   op0=OP.mult, op1=OP.subtract)
                    nc.scalar.activation(var, var, AF.Sqrt, bias=eps128[0:1, :])
                    rstd = pt4.tile([1, 512], F32, tag="rstd")
                    nc.vector.reciprocal_approx_fast(out=rstd, in_=var)
                    arow = pt4.tile([1, 512], F32, tag="arow")
                    nc.vector.tensor_tensor(arow, rstd, s1row[0:1, sl], OP.mult)
                    arow_bf = pt4.tile([1, 512], BF16, tag="arow_bf")
                    nc.vector.tensor_copy(arow_bf, arow)
                    crow_bf = pt4.tile([1, 512], BF16, tag="crow_bf")
                    nc.vector.tensor_tensor(crow_bf, mu, arow, OP.mult)
                    psa = pAps.tile([P, 512], F32, tag="ps_a")
                    nc.tensor.matmul(psa, ones_row, arow_bf, start=True, stop=True)
                    psc = pAps.tile([P, 512], F32, tag="ps_c")
                    nc.tensor.matmul(psc, ones_row, crow_bf, start=True, stop=True)
                    if f["ln1b"]:
                        s1_bf = pt4.tile([1, 512], BF16, tag="s1_bf")
                        nc.vector.tensor_copy(s1_bf, s1row[0:1, sl])
                        pss1 = pAps.tile([P, 512], F32, tag="ps_s1")
                        nc.tensor.matmul(pss1, ones_row, s1_bf, start=True, stop=True)
                    for mc in range(HC):
                        y = xTf[:, mc, sl]
                        nc.vector.tensor_tensor(y, y, psa, OP.mult)
                        nc.vector.tensor_tensor(y, y, psc, OP.subtract)
                        if f["ln1w"]:
                            nc.vector.tensor_scalar(y, y, ln1w_sb[:, mc:mc + 1], None,
                                                    op0=OP.mult)
                        if f["ln1b"]:
                            bs = pt4.tile([P, 512], F32, tag="bs")
                            nc.vector.tensor_scalar(bs, pss1, ln1b_sb[:, mc:mc + 1],
                                                    None, op0=OP.mult)
                            nc.vector.tensor_tensor(y, y, bs, OP.add)
                        if f["beta1"]:
                            nc.vector.tensor_scalar(y, y, sct["beta1"], None, op0=OP.add)
                        nc.scalar.activation(x1f8[:, mc, sl], y, AF.Identity)

                # gate first layer (runs while LN1 of the second q-tile drains)
                for qt in range(QT):
                    sl = slice(qt * 512, (qt + 1) * 512)
                    for mc in range(GC):
                        wt = pw.tile([P, HC, P], F8, tag="wproj", bufs=3)
                        nc.sync.dma_start(out=wt, in_=wT3(g1)[:, :, mc * P:(mc + 1) * P])
                        ps = pAps.tile([P, 512], F32, tag="ps_g1", bufs=g1_bufs)
                        for g in range(HC // 2):
                            nc.tensor.matmul(ps, wt[:, 2 * g:2 * g + 2, :],
                                             x1f8[:, 2 * g:2 * g + 2, sl],
                                             perf_mode=DR,
                                             start=(g == 0), stop=(g == HC // 2 - 1))
                        nc.scalar.activation(rT[:, mc, sl], ps, AF.Relu,
                                             bias=gb1_sb[:, mc:mc + 1], scale=WS)

              y1f8 = x1f8  # fp8 y1; xTf holds f32 y1

              # ---------------- P6: gate2 + FFN + gated mix; P7 LN2 --------
              # hT for a full ffn-half lives in the dead KTs/ctxT slabs; g2,
              # W1-build, W2-accumulate and P7 transposes all rotate through
              # the same four psum tags (2 banks each) so W1/W2 stream once.
              accf = trunk.tile([P, HC, S], F32, tag="f4c", name="accf")
              if True:
                with tc.tile_pool(name="pt7", bufs=2) as pt7, \
                     tc.tile_pool(name="pCps", bufs=1, space="PSUM") as pCps:
                    psk = [0]

                    def accps(shape):
                        t = pCps.tile(shape, F32, tag=f"ps_acc{psk[0] % 4}",
                                      name=f"psacc{psk[0] % 4}")
                        psk[0] += 1
                        return t

                    for qt in range(QT):
                        for mc in range(HC):
                            wt = pw.tile([P, GC, P], F8, tag="wg2", bufs=3)
                            nc.sync.dma_start(out=wt,
                                              in_=g2.rearrange("(gc p) o -> p gc o", p=P)
                                              [:, :, mc * P:(mc + 1) * P])
                            ps = accps([P, 512])
                            for g in range(GC // 2):
                                nc.tensor.matmul(ps, wt[:, 2 * g:2 * g + 2, :],
                                                 rT[:, 2 * g:2 * g + 2,
                                                    qt * 512:(qt + 1) * 512],
                                                 perf_mode=DR,
                                                 start=(g == 0), stop=(g == GC // 2 - 1))
                            nc.scalar.activation(gT[:, mc, qt * 512:(qt + 1) * 512], ps,
                                                 AF.Sigmoid, bias=gb2_sb[:, mc:mc + 1],
                                                 scale=WS)
                    for half in range(2):
                        hA = trunk.tile([P, 8, S], F8, tag="bf2b", name="hA")
                        hB = trunk.tile([P, 8, S], F8, tag="bf2d", name="hB")

                        def hsl(c, qsl=slice(None)):
                            return (hA[:, c, qsl] if c < 8 else hB[:, c - 8, qsl])

                        def hsl2(c, qsl):  # [P, 2, n] slice for DoubleRow rhs
                            return (hA[:, c:c + 2, qsl] if c < 8
                                    else hB[:, c - 8:c - 6, qsl])

                        for c in range(16):
                            cg = half * 16 + c
                            wt = pw.tile([P, HC, P], F8, tag="wproj", bufs=3)
                            nc.sync.dma_start(out=wt,
                                              in_=wT3(w1)[:, :, cg * P:(cg + 1) * P])
                            psh = accps([P, S])
                            for qt in range(QT):
                                for g in range(HC // 2):
                                    nc.tensor.matmul(
                                        psh[:, qt * 512:(qt + 1) * 512],
                                        wt[:, 2 * g:2 * g + 2, :],
                                        y1f8[:, 2 * g:2 * g + 2,
                                             qt * 512:(qt + 1) * 512],
                                        perf_mode=DR,
                                        start=(g == 0), stop=(g == HC // 2 - 1))
                            nc.scalar.activation(hsl(c), psh, AF.Gelu,
                                                 bias=b1_sb[:, cg:cg + 1], scale=WS)
                        for oh in range(2):
                            accs = [accps([P, S]) for mc in range(4)]
                            for cp in range(8):
                                cg = half * 16 + 2 * cp
                                wt = pw.tile([P, 2, 512], F8, tag="w2", bufs=6)
                                nc.sync.dma_start(
                                    out=wt,
                                    in_=w2.rearrange("(cc p) o -> p cc o", p=P)
                                    [:, cg:cg + 2, oh * 512:(oh + 1) * 512])
                                for mc in range(4):
                                    for qt in range(QT):
                                        nc.tensor.matmul(
                                            accs[mc][:, qt * 512:(qt + 1) * 512],
                                            wt[:, :, mc * P:(mc + 1) * P],
                                            hsl2(2 * cp, slice(qt * 512, (qt + 1) * 512)),
                                            perf_mode=DR,
                                            start=(cp == 0), stop=(cp == 7))
                            for mc in range(4):
                                mcg = oh * 4 + mc
                                for qt in range(QT):
                                    a = accf[:, mcg, qt * 512:(qt + 1) * 512]
                                    psl = accs[mc][:, qt * 512:(qt + 1) * 512]
                                    y = xTf[:, mcg, qt * 512:(qt + 1) * 512]
                                    if half == 0:
                                        # store (ffn_half0 - y1): saves a pass later
                                        nc.vector.scalar_tensor_tensor(
                                            a, psl, WS, y, op0=OP.mult, op1=OP.subtract)
                                    else:
                                        nc.vector.scalar_tensor_tensor(
                                            a, psl, WS, a, op0=OP.mult, op1=OP.add)
                                        if f["b2"]:
                                            nc.vector.tensor_scalar(
                                                a, a, b2_sb[:, mcg:mcg + 1], None,
                                                op0=OP.add)
                                        g = gT[:, mcg, qt * 512:(qt + 1) * 512]
                                        nc.vector.tensor_tensor(a, a, g, OP.mult)
                                        nc.vector.scalar_tensor_tensor(
                                            a, y, 2.0, a, op0=OP.mult, op1=OP.add)

                    # ---------------- P7: LN2 (N-layout) + output ------------
                    for qc in range(HC):
                        xt = pt7.tile([P, H], F32, tag="x2")
                        for hc in range(HC):
                            pst = accps([P, P])
                            nc.tensor.transpose(pst, accf[:, hc, qc * P:(qc + 1) * P],
                                                identf)
                            nc.scalar.activation(xt[:, hc * P:(hc + 1) * P], pst,
                                                 AF.Identity)
                        stats = pt7.tile([P, 2, nc.vector.BN_STATS_DIM], F32, tag="stats")
                        for sg in range(2):
                            nc.vector.bn_stats(stats[:, sg, :],
                                               xt[:, sg * 512:(sg + 1) * 512])
                        mv = pt7.tile([P, nc.vector.BN_AGGR_DIM], F32, tag="mv")
                        nc.vector.bn_aggr(mv, stats)
                        sd = pt7.tile([P, 1], F32, tag="sd")
                        nc.scalar.activation(sd, mv[:, 1:2], AF.Sqrt, bias=eps128)
                        rstd2 = pt7.tile([P, 1], F32, tag="rstd2")
                        nc.vector.reciprocal(rstd2, sd)
                        a2 = pt7.tile([P, 1], F32, tag="a2")
                        nc.vector.tensor_tensor(a2, rstd2, s2_np[:, qc:qc + 1], OP.mult)
                        ot = pt7.tile([P, H], F32, tag="ot")
                        nc.vector.tensor_scalar(ot, xt, mv[:, 0:1], a2,
                                                op0=OP.subtract, op1=OP.mult)
                        if f["ln2w"]:
                            nc.vector.tensor_tensor(ot, ot, ln2w_bc, OP.mult)
                        if f["ln2b"]:
                            bs2 = pt7.tile([P, H], F32, tag="bs2")
                            nc.vector.tensor_scalar(bs2, ln2b_bc, s2_np[:, qc:qc + 1],
                                                    None, op0=OP.mult)
                            nc.vector.tensor_tensor(ot, ot, bs2, OP.add)
                        if f["beta2"]:
                            nc.vector.tensor_scalar(ot, ot, sct["beta2"], None,
                                                    op0=OP.add)
                        nc.sync.dma_start(out=out[qc * P:(qc + 1) * P, :], in_=ot)

    nc.compile()
    return nc


def _prep(inputs):
    """Host-side prep: per-core in_maps (DP over batch) + build flags."""
    f8 = ml_dtypes.float8_e4m3
    x = np.asarray(inputs["x"], np.float32)
    volat = np.asarray(inputs["volatility"], np.float32)

    def w_f8(name):
        w = np.asarray(inputs[name], np.float32) * WSC
        return np.ascontiguousarray(np.clip(w, -240.0, 240.0).astype(f8))

    shared = {
        "wq": w_f8("Wq"), "wk": w_f8("Wk"), "wv": w_f8("Wv"), "wo": w_f8("Wo"),
        "w1": w_f8("ffn_w1"), "w2": w_f8("ffn_w2"),
        "g1": w_f8("gate_w1"), "g2": w_f8("gate_w2"),
    }
    for name, key in (("bq", "bq"), ("bk", "bk"), ("bv", "bv"), ("bo", "bo"),
                      ("b1", "ffn_b1"), ("b2", "ffn_b2"),
                      ("gb1", "gate_b1"), ("gb2", "gate_b2"),
                      ("ln1w", "ln1_w"), ("ln1b", "ln1_b"),
                      ("ln2w", "ln2_w"), ("ln2b", "ln2_b")):
        shared[name] = np.ascontiguousarray(np.asarray(inputs[key], np.float32))
    for name, key in (("gamma1", "gamma1"), ("beta1", "beta1"),
                      ("vs1w", "vs1_w"), ("vs1b", "vs1_b"),
                      ("gamma2", "gamma2"), ("beta2", "beta2"),
                      ("vs2w", "vs2_w"), ("vs2b", "vs2_b")):
        shared[name] = np.asarray(inputs[key], np.float32).reshape(1)

    flags = (
        ("bv", bool(np.any(shared["bv"]))),
        ("bo", bool(np.any(shared["bo"]))),
        ("b2", bool(np.any(shared["b2"]))),
        ("ln1w", bool(np.any(shared["ln1w"] != 1.0))),
        ("ln1b", bool(np.any(shared["ln1b"]))),
        ("beta1", bool(shared["beta1"][0] != 0.0)),
        ("ln2w", bool(np.any(shared["ln2w"] != 1.0))),
        ("ln2b", bool(np.any(shared["ln2b"]))),
        ("beta2", bool(shared["beta2"][0] != 0.0)),
    )

    in_maps = []
    for b in range(B):
        m = dict(shared)
        xt = np.ascontiguousarray(x[b].T)
        m["xt8"] = np.ascontiguousarray(np.clip(xt, -240.0, 240.0).astype(f8))
        m["xtf"] = xt
        m["vol"] = np.ascontiguousarray(volat[b])
        in_maps.append(m)
    return in_maps, flags


def _run(inputs, trace=False):
    in_maps, flags = _prep(inputs)
    if flags not in _BUILD_CACHE:
        _BUILD_CACHE[flags] = _build(flags)
    nc = _BUILD_CACHE[flags]
    res = run_bass_kernel_spmd(nc, in_maps, core_ids=list(range(B)), trace=trace)
    outs = np.stack([res.results[b]["out"] for b in range(B)], axis=0)
    return outs.astype(np.float32), res


def kernel(**inputs) -> np.ndarray:
    out, _ = _run(inputs, trace=False)
    return out
